# revision 7
# baseline (speedup 1.0000x reference)
"""Trainium2 Bass kernel for nn_MemoryEfficientAttention (full MHA).

Reference computation (fp32):
    q = split_heads(x @ Wq.T + bq); k, v likewise
    attn = softmax(q @ k.T / sqrt(64))
    out = merge_heads(attn @ v) @ Wo.T + bo

Shapes: B=2, S=4096, D=1024, H=16, head_dim=64.

Sharding across 8 NeuronCores (Megatron attention sharding):
  - 2 heads per core (= 128 of the 1024 projection dims, contiguous slice).
  - Q/K/V projections column-parallel, output projection row-parallel;
    the 8 per-core partial outputs are summed on the host (+ bo).

Per-core kernel (flash-attention style, nothing S^2-sized touches HBM):
  Phase 1: qT/kT/vT = W_c @ x.T + b_c   as fp32r matmuls ([128, S] layouts),
           V additionally PE-transposed to natural [S, 128] layout with an
           interleaved ones-column per head (v_aug).
  Phase 2: per (batch, q-chunk of 512): loop over 32 key tiles:
           scoresT[kpos, q] for both heads (row-packed in the PE array),
           exp on ScalarE (no max subtraction -- scores are bounded, the
           softmax is mathematically identical), PV matmul with M=65: rows
           0..63 accumulate v.T @ expT, row 64 accumulates the softmax
           denominator. Normalize via reciprocal + DMA broadcast.
  Phase 3: out[s, :] = attnT_c.T @ WoT_c  (natural layout, clean DMA out).
"""

import sys

if "/opt/trn_rl_repo" not in sys.path:
    sys.path.insert(0, "/opt/trn_rl_repo")

import numpy as np

B = 2
S_FULL = 4096
D = 1024
H = 16
HD = 64
NCORES = 8
DC = 128          # head dims per core (2 heads x 64)
SCALE = 1.0 / 8.0  # 1/sqrt(64)


def build_kernel(S=S_FULL):
    """Build the per-core Bass program. Returns the compiled Bacc object."""
    import concourse.bacc as bacc
    import concourse.tile as tile
    from concourse import mybir

    f32 = mybir.dt.float32
    f32r = mybir.dt.float32r
    AF = mybir.ActivationFunctionType

    KT = D // 128       # k-tiles over the projection contraction dim
    SQ = 512            # q-chunk size
    NQC = S // SQ       # q chunks per batch
    NKT = S // 128      # key tiles per batch
    NM = S // 512       # x chunks for projections

    nc = bacc.Bacc("TRN2", target_bir_lowering=False, debug=False,
                   num_devices=NCORES)

    xT = nc.dram_tensor("xT", [B, D, S], f32r, kind="ExternalInput").ap()
    wqT = nc.dram_tensor("wqT", [D, DC], f32r, kind="ExternalInput").ap()
    wkT = nc.dram_tensor("wkT", [D, DC], f32r, kind="ExternalInput").ap()
    wvT = nc.dram_tensor("wvT", [D, DC], f32r, kind="ExternalInput").ap()
    woT = nc.dram_tensor("woT", [DC, D], f32r, kind="ExternalInput").ap()
    bq = nc.dram_tensor("bq", [DC], f32, kind="ExternalInput").ap()
    bk = nc.dram_tensor("bk", [DC], f32, kind="ExternalInput").ap()
    bv = nc.dram_tensor("bv", [DC], f32, kind="ExternalInput").ap()
    ident = nc.dram_tensor("ident", [128, 128], f32r, kind="ExternalInput").ap()
    ones = nc.dram_tensor("ones", [128, 32], f32r, kind="ExternalInput").ap()
    part = nc.dram_tensor("part", [B, S, D], f32, kind="ExternalOutput").ap()

    with tile.TileContext(nc) as tc:
        with (
            tc.tile_pool(name="consts", bufs=1) as consts,
            tc.tile_pool(name="xt", bufs=2) as xt_pool,
            tc.tile_pool(name="qkv", bufs=1) as qkv_pool,
            tc.tile_pool(name="exp", bufs=3) as exp_pool,
            tc.tile_pool(name="att", bufs=1) as att_pool,
            tc.tile_pool(name="small", bufs=4) as small_pool,
            tc.tile_pool(name="outs", bufs=3) as out_pool,
            tc.tile_pool(name="bounce", bufs=4, space="DRAM") as dram_pool,
            tc.tile_pool(name="ps_proj", bufs=2, space="PSUM") as ps_proj,
            tc.tile_pool(name="ps_scores", bufs=2, space="PSUM") as ps_scores,
            tc.tile_pool(name="ps_acc", bufs=2, space="PSUM") as ps_acc,
        ):
            # ---- constants ----
            wq_sb = consts.tile([128, KT, DC], f32r)
            wk_sb = consts.tile([128, KT, DC], f32r)
            wv_sb = consts.tile([128, KT, DC], f32r)
            wo_sb = consts.tile([128, D], f32r)
            bq_sb = consts.tile([128, 1], f32)
            bk_sb = consts.tile([128, 1], f32)
            bv_sb = consts.tile([128, 1], f32)
            id_sb = consts.tile([128, 128], f32r)

            for w_sb, w_dram in ((wq_sb, wqT), (wk_sb, wkT), (wv_sb, wvT)):
                nc.sync.dma_start(
                    out=w_sb[:],
                    in_=w_dram.rearrange("(kt p) m -> p kt m", p=128),
                )
            nc.sync.dma_start(out=wo_sb[:], in_=woT)
            for b_sb, b_dram in ((bq_sb, bq), (bk_sb, bk), (bv_sb, bv)):
                nc.sync.dma_start(out=b_sb[:], in_=b_dram.rearrange("(p o) -> p o", o=1))
            nc.sync.dma_start(out=id_sb[:], in_=ident)

            for b in range(B):
                # ================= Phase 1: projections =================
                qT_sb = qkv_pool.tile([128, S], f32r, tag="qT")
                kT_sb = qkv_pool.tile([128, S], f32r, tag="kT")
                vT_sb = qkv_pool.tile([128, S], f32r, tag="vT")
                # v natural layout, per key-tile: [vA(64) | 1 | vB(64) | 1]
                v_sb = qkv_pool.tile([128, NKT, 130], f32r, tag="v")

                for m in range(NM):
                    xt = xt_pool.tile([128, KT, 512], f32r)
                    nc.sync.dma_start(
                        out=xt[:],
                        in_=xT[b][:, m * 512:(m + 1) * 512].rearrange(
                            "(kt p) s -> p kt s", p=128),
                    )
                    for w_sb, b_sb, dst in (
                        (wq_sb, bq_sb, qT_sb),
                        (wk_sb, bk_sb, kT_sb),
                        (wv_sb, bv_sb, vT_sb),
                    ):
                        ps = ps_proj.tile([128, 512], f32, tag="proj")
                        for j in range(KT):
                            nc.tensor.matmul(
                                ps[:],
                                lhsT=w_sb[:, j, :],
                                rhs=xt[:, j, :],
                                start=(j == 0),
                                stop=(j == KT - 1),
                            )
                        nc.vector.tensor_scalar_add(
                            dst[:, m * 512:(m + 1) * 512], ps[:], b_sb[:],
                        )

                # V: transpose to natural layout + ones columns
                ones_src = ones[:, 0:NKT].rearrange("p (t o) -> p t o", o=1)
                nc.sync.dma_start(out=v_sb[:, :, 64:65], in_=ones_src)
                nc.sync.dma_start(out=v_sb[:, :, 129:130], in_=ones_src)
                for t in range(NKT):
                    pst = ps_proj.tile([128, 512], f32r, tag="proj")
                    nc.tensor.transpose(
                        pst[:, 0:128], vT_sb[:, t * 128:(t + 1) * 128], id_sb[:],
                    )
                    nc.vector.tensor_copy(v_sb[:, t, 0:64], pst[:, 0:64])
                    nc.vector.tensor_copy(v_sb[:, t, 65:129], pst[:, 64:128])

                # ================= Phase 2: attention =================
                attT_sb = att_pool.tile([128, S], f32r, tag="attT")
                for qc in range(NQC):
                    q0, q1 = qc * SQ, (qc + 1) * SQ
                    acc_a = ps_acc.tile([128, SQ], f32, tag="acc")
                    acc_b = ps_acc.tile([128, SQ], f32, tag="acc")
                    accs = [acc_a, acc_b]
                    for j in range(NKT):
                        k0, k1 = j * 128, (j + 1) * 128
                        pss = ps_scores.tile([128, 2 * SQ], f32, tag="scores")
                        for hh in range(2):
                            nc.tensor.matmul(
                                pss[:, hh * SQ:(hh + 1) * SQ],
                                lhsT=kT_sb[hh * 64:(hh + 1) * 64, k0:k1],
                                rhs=qT_sb[hh * 64:(hh + 1) * 64, q0:q1],
                                start=True, stop=True,
                            )
                        ex = exp_pool.tile([128, 2 * SQ], f32r, tag="exp")
                        nc.scalar.activation(ex[:], pss[:], AF.Exp, scale=SCALE)
                        for hh in range(2):
                            nc.tensor.matmul(
                                accs[hh][0:65, :],
                                lhsT=v_sb[:, j, hh * 65:(hh + 1) * 65],
                                rhs=ex[:, hh * SQ:(hh + 1) * SQ],
                                start=(j == 0), stop=(j == NKT - 1),
                            )
                    # normalize: attT[hh rows, qchunk] = acc[0:64] * (1/denom)
                    for hh in range(2):
                        rc = small_pool.tile([1, SQ], f32, tag="recip")
                        nc.vector.reciprocal(rc[:], accs[hh][64:65, :])
                        rc_dram = dram_pool.tile([1, SQ], f32, tag="rcd")
                        nc.sync.dma_start(out=rc_dram[:], in_=rc[:])
                        bc = small_pool.tile([64, SQ], f32, tag="bcast")
                        rd = rc_dram[:]
                        bcast_src = rd.__class__(
                            tensor=rd.tensor, offset=rd.offset,
                            ap=[[0, 64]] + list(rd.ap)[1:],
                        )
                        nc.sync.dma_start(out=bc[:], in_=bcast_src)
                        nc.vector.tensor_mul(
                            attT_sb[hh * 64:(hh + 1) * 64, q0:q1],
                            accs[hh][0:64, :],
                            bc[:],
                        )

                # ================= Phase 3: output projection =================
                for st in range(S // 128):
                    s0, s1 = st * 128, (st + 1) * 128
                    for oc in range(D // 512):
                        pso = ps_acc.tile([128, 512], f32, tag="acc")
                        nc.tensor.matmul(
                            pso[:],
                            lhsT=attT_sb[:, s0:s1],
                            rhs=wo_sb[:, oc * 512:(oc + 1) * 512],
                            start=True, stop=True,
                        )
                        ob = out_pool.tile([128, 512], f32, tag="ob")
                        nc.vector.tensor_copy(ob[:], pso[:])
                        nc.sync.dma_start(
                            out=part[b, s0:s1, oc * 512:(oc + 1) * 512],
                            in_=ob[:],
                        )

    nc.compile()
    return nc


def shard_inputs(x, Wq, bq, Wk, bk, Wv, bv, Wo, bo, S=S_FULL):
    """Host-side sharding: returns list of 8 per-core input dicts."""
    x = np.asarray(x, dtype=np.float32)
    xT = np.ascontiguousarray(x.transpose(0, 2, 1))  # [B, D, S]
    ident = np.eye(128, dtype=np.float32)
    in_maps = []
    for c in range(NCORES):
        sl = slice(c * DC, (c + 1) * DC)
        in_maps.append({
            "xT": xT,
            "wqT": np.ascontiguousarray(np.asarray(Wq)[sl, :].T, dtype=np.float32),
            "wkT": np.ascontiguousarray(np.asarray(Wk)[sl, :].T, dtype=np.float32),
            "wvT": np.ascontiguousarray(np.asarray(Wv)[sl, :].T, dtype=np.float32),
            "woT": np.ascontiguousarray(np.asarray(Wo)[:, sl].T, dtype=np.float32),
            "bq": np.ascontiguousarray(np.asarray(bq)[sl], dtype=np.float32),
            "bk": np.ascontiguousarray(np.asarray(bk)[sl], dtype=np.float32),
            "bv": np.ascontiguousarray(np.asarray(bv)[sl], dtype=np.float32),
            "ident": ident,
            "ones": np.ones((128, 32), dtype=np.float32),
        })
    return in_maps


_NC_CACHE = {}


def _get_nc(S=S_FULL):
    if S not in _NC_CACHE:
        _NC_CACHE[S] = build_kernel(S)
    return _NC_CACHE[S]


def kernel(x, Wq, bq, Wk, bk, Wv, bv, Wo, bo, _trace=False, _trace_cores=None):
    from concourse import bass_utils

    nc = _get_nc(S_FULL)
    in_maps = shard_inputs(x, Wq, bq, Wk, bk, Wv, bv, Wo, bo)
    kwargs = {}
    if _trace:
        kwargs = dict(trace=True, trace_cores=_trace_cores or [0])
    res = bass_utils.run_bass_kernel_spmd(
        nc, in_maps, core_ids=list(range(NCORES)), **kwargs)
    out = np.zeros((B, S_FULL, D), dtype=np.float32)
    for c in range(NCORES):
        out += res.results[c]["part"]
    out += np.asarray(bo, dtype=np.float32)[None, None, :]
    if _trace:
        kernel._last_results = res
    return out


# revision 10
# speedup vs baseline: 1.2206x; 1.2206x over previous
"""Trainium2 Bass kernel for nn_MemoryEfficientAttention (full MHA).

Reference computation (fp32):
    q = split_heads(x @ Wq.T + bq); k, v likewise
    attn = softmax(q @ k.T / sqrt(64))
    out = merge_heads(attn @ v) @ Wo.T + bo

Shapes: B=2, S=4096, D=1024, H=16, head_dim=64.

Sharding across 8 NeuronCores (Megatron attention sharding):
  - 2 heads per core (= 128 of the 1024 projection dims, contiguous slice).
  - Q/K/V projections column-parallel, output projection row-parallel;
    the 8 per-core partial outputs are summed on the host (+ bo).

Per-core kernel (flash-attention style, nothing S^2-sized touches HBM):
  Phase 1: qT/kT/vT = W_c @ x.T + b_c   as fp32r matmuls ([128, S] layouts),
           V additionally PE-transposed to natural [S, 128] layout with an
           interleaved ones-column per head (v_aug).
  Phase 2: per (batch, q-chunk of 512): loop over 32 key tiles:
           scoresT[kpos, q] for both heads (row-packed in the PE array),
           exp on ScalarE (no max subtraction -- scores are bounded, the
           softmax is mathematically identical), PV matmul with M=65: rows
           0..63 accumulate v.T @ expT, row 64 accumulates the softmax
           denominator. Normalize via reciprocal + DMA broadcast.
  Phase 3: out[s, :] = attnT_c.T @ WoT_c  (natural layout, clean DMA out).
"""

import sys

if "/opt/trn_rl_repo" not in sys.path:
    sys.path.insert(0, "/opt/trn_rl_repo")

import numpy as np

B = 2
S_FULL = 4096
D = 1024
H = 16
HD = 64
NCORES = 8
DC = 128          # head dims per core (2 heads x 64)
SCALE = 1.0 / 8.0  # 1/sqrt(64)


def build_kernel(S=S_FULL):
    """Build the per-core Bass program. Returns the compiled Bacc object."""
    import concourse.bacc as bacc
    import concourse.tile as tile
    from concourse import mybir

    f32 = mybir.dt.float32
    f32r = mybir.dt.float32r
    f16 = mybir.dt.float16
    AF = mybir.ActivationFunctionType

    KT = D // 128       # k-tiles over the projection contraction dim
    SQ = 512            # q-chunk size
    NQC = S // SQ       # q chunks per batch
    NKT = S // 128      # key tiles per batch
    NM = S // 512       # x chunks for projections

    nc = bacc.Bacc("TRN2", target_bir_lowering=False, debug=False,
                   num_devices=NCORES)

    xT = nc.dram_tensor("xT", [B, D, S], f32r, kind="ExternalInput").ap()
    wqT = nc.dram_tensor("wqT", [D, DC], f32r, kind="ExternalInput").ap()
    wkT = nc.dram_tensor("wkT", [D, DC], f32r, kind="ExternalInput").ap()
    wvT = nc.dram_tensor("wvT", [D, DC], f32r, kind="ExternalInput").ap()
    woT = nc.dram_tensor("woT", [DC, D], f32r, kind="ExternalInput").ap()
    bq = nc.dram_tensor("bq", [DC], f32, kind="ExternalInput").ap()
    bk = nc.dram_tensor("bk", [DC], f32, kind="ExternalInput").ap()
    bv = nc.dram_tensor("bv", [DC], f32, kind="ExternalInput").ap()
    ident = nc.dram_tensor("ident", [128, 128], f32r, kind="ExternalInput").ap()
    ones = nc.dram_tensor("ones", [128, 32], f16, kind="ExternalInput").ap()
    part = nc.dram_tensor("part", [B, S, D], f32, kind="ExternalOutput").ap()

    with tile.TileContext(nc) as tc:
        with (
            tc.tile_pool(name="consts", bufs=1) as consts,
            tc.tile_pool(name="xt", bufs=2) as xt_pool,
            tc.tile_pool(name="qkv", bufs=1) as qkv_pool,
            tc.tile_pool(name="exp", bufs=3) as exp_pool,
            tc.tile_pool(name="att", bufs=1) as att_pool,
            tc.tile_pool(name="small", bufs=4) as small_pool,
            tc.tile_pool(name="outs", bufs=3) as out_pool,
            tc.tile_pool(name="bounce", bufs=4, space="DRAM") as dram_pool,
            tc.tile_pool(name="ps_proj", bufs=2, space="PSUM") as ps_proj,
            tc.tile_pool(name="ps_scores", bufs=2, space="PSUM") as ps_scores,
            tc.tile_pool(name="ps_acc", bufs=2, space="PSUM") as ps_acc,
        ):
            # ---- constants ----
            wq_sb = consts.tile([128, KT, DC], f32r)
            wk_sb = consts.tile([128, KT, DC], f32r)
            wv_sb = consts.tile([128, KT, DC], f32r)
            wo_sb = consts.tile([128, D], f32r)
            bq_sb = consts.tile([128, 1], f32)
            bk_sb = consts.tile([128, 1], f32)
            bv_sb = consts.tile([128, 1], f32)
            id_sb = consts.tile([128, 128], f32r)

            for w_sb, w_dram in ((wq_sb, wqT), (wk_sb, wkT), (wv_sb, wvT)):
                nc.sync.dma_start(
                    out=w_sb[:],
                    in_=w_dram.rearrange("(kt p) m -> p kt m", p=128),
                )
            nc.sync.dma_start(out=wo_sb[:], in_=woT)
            for b_sb, b_dram in ((bq_sb, bq), (bk_sb, bk), (bv_sb, bv)):
                nc.sync.dma_start(out=b_sb[:], in_=b_dram.rearrange("(p o) -> p o", o=1))
            nc.sync.dma_start(out=id_sb[:], in_=ident)

            for b in range(B):
                # ================= Phase 1: projections =================
                qT_sb = qkv_pool.tile([128, S], f16, tag="qT")
                kT_sb = qkv_pool.tile([128, S], f16, tag="kT")
                vT_sb = qkv_pool.tile([128, S], f32r, tag="vT")
                # v natural layout, per key-tile: [vA(64) | 1 | vB(64) | 1]
                v_sb = qkv_pool.tile([128, NKT, 130], f16, tag="v")

                for m in range(NM):
                    xt = xt_pool.tile([128, KT, 512], f32r)
                    nc.sync.dma_start(
                        out=xt[:],
                        in_=xT[b][:, m * 512:(m + 1) * 512].rearrange(
                            "(kt p) s -> p kt s", p=128),
                    )
                    for w_sb, b_sb, dst in (
                        (wq_sb, bq_sb, qT_sb),
                        (wk_sb, bk_sb, kT_sb),
                        (wv_sb, bv_sb, vT_sb),
                    ):
                        ps = ps_proj.tile([128, 512], f32, tag="proj")
                        for j in range(KT):
                            nc.tensor.matmul(
                                ps[:],
                                lhsT=w_sb[:, j, :],
                                rhs=xt[:, j, :],
                                start=(j == 0),
                                stop=(j == KT - 1),
                            )
                        nc.vector.tensor_scalar_add(
                            dst[:, m * 512:(m + 1) * 512], ps[:], b_sb[:],
                        )

                # V: transpose to natural layout + ones columns
                ones_src = ones[:, 0:NKT].rearrange("p (t o) -> p t o", o=1)
                nc.sync.dma_start(out=v_sb[:, :, 64:65], in_=ones_src)
                nc.sync.dma_start(out=v_sb[:, :, 129:130], in_=ones_src)
                for t in range(NKT):
                    pst = ps_proj.tile([128, 512], f32r, tag="proj")
                    nc.tensor.transpose(
                        pst[:, 0:128], vT_sb[:, t * 128:(t + 1) * 128], id_sb[:],
                    )
                    nc.vector.tensor_copy(v_sb[:, t, 0:64], pst[:, 0:64])
                    nc.vector.tensor_copy(v_sb[:, t, 65:129], pst[:, 64:128])

                # ================= Phase 2: attention =================
                attT_sb = att_pool.tile([128, S], f32r, tag="attT")
                for qc in range(NQC):
                    q0, q1 = qc * SQ, (qc + 1) * SQ
                    acc_a = ps_acc.tile([128, SQ], f32, tag="acc")
                    acc_b = ps_acc.tile([128, SQ], f32, tag="acc")
                    accs = [acc_a, acc_b]
                    for j in range(NKT):
                        k0, k1 = j * 128, (j + 1) * 128
                        pss = ps_scores.tile([128, 2 * SQ], f32, tag="scores")
                        for hh in range(2):
                            nc.tensor.matmul(
                                pss[:, hh * SQ:(hh + 1) * SQ],
                                lhsT=kT_sb[hh * 64:(hh + 1) * 64, k0:k1],
                                rhs=qT_sb[hh * 64:(hh + 1) * 64, q0:q1],
                                start=True, stop=True,
                            )
                        ex = exp_pool.tile([128, 2 * SQ], f16, tag="exp")
                        nc.scalar.activation(ex[:], pss[:], AF.Exp, scale=SCALE)
                        for hh in range(2):
                            nc.tensor.matmul(
                                accs[hh][0:65, :],
                                lhsT=v_sb[:, j, hh * 65:(hh + 1) * 65],
                                rhs=ex[:, hh * SQ:(hh + 1) * SQ],
                                start=(j == 0), stop=(j == NKT - 1),
                            )
                    # normalize: attT[hh rows, qchunk] = acc[0:64] * (1/denom)
                    for hh in range(2):
                        dn_sb = small_pool.tile([1, SQ], f32, tag="dn")
                        nc.vector.tensor_copy(dn_sb[:], accs[hh][64:65, :])
                        dn_dram = dram_pool.tile([1, SQ], f32, tag="rcd")
                        nc.sync.dma_start(out=dn_dram[:], in_=dn_sb[:])
                        bc = small_pool.tile([64, SQ], f32, tag="bcast")
                        rd = dn_dram[:]
                        bcast_src = rd.__class__(
                            tensor=rd.tensor, offset=rd.offset,
                            ap=[[0, 64]] + list(rd.ap)[1:],
                        )
                        nc.sync.dma_start(out=bc[:], in_=bcast_src)
                        rc = small_pool.tile([64, SQ], f32, tag="recip")
                        nc.vector.reciprocal(rc[:], bc[:])
                        nc.vector.tensor_mul(
                            attT_sb[hh * 64:(hh + 1) * 64, q0:q1],
                            accs[hh][0:64, :],
                            rc[:],
                        )

                # ================= Phase 3: output projection =================
                for st in range(S // 128):
                    s0, s1 = st * 128, (st + 1) * 128
                    for oc in range(D // 512):
                        pso = ps_acc.tile([128, 512], f32, tag="acc")
                        nc.tensor.matmul(
                            pso[:],
                            lhsT=attT_sb[:, s0:s1],
                            rhs=wo_sb[:, oc * 512:(oc + 1) * 512],
                            start=True, stop=True,
                        )
                        ob = out_pool.tile([128, 512], f32, tag="ob")
                        nc.vector.tensor_copy(ob[:], pso[:])
                        nc.sync.dma_start(
                            out=part[b, s0:s1, oc * 512:(oc + 1) * 512],
                            in_=ob[:],
                        )

    nc.compile()
    return nc


def shard_inputs(x, Wq, bq, Wk, bk, Wv, bv, Wo, bo, S=S_FULL):
    """Host-side sharding: returns list of 8 per-core input dicts."""
    x = np.asarray(x, dtype=np.float32)
    xT = np.ascontiguousarray(x.transpose(0, 2, 1))  # [B, D, S]
    ident = np.eye(128, dtype=np.float32)
    in_maps = []
    for c in range(NCORES):
        sl = slice(c * DC, (c + 1) * DC)
        in_maps.append({
            "xT": xT,
            "wqT": np.ascontiguousarray(np.asarray(Wq)[sl, :].T, dtype=np.float32),
            "wkT": np.ascontiguousarray(np.asarray(Wk)[sl, :].T, dtype=np.float32),
            "wvT": np.ascontiguousarray(np.asarray(Wv)[sl, :].T, dtype=np.float32),
            "woT": np.ascontiguousarray(np.asarray(Wo)[:, sl].T, dtype=np.float32),
            "bq": np.ascontiguousarray(np.asarray(bq)[sl], dtype=np.float32),
            "bk": np.ascontiguousarray(np.asarray(bk)[sl], dtype=np.float32),
            "bv": np.ascontiguousarray(np.asarray(bv)[sl], dtype=np.float32),
            "ident": ident,
            "ones": np.ones((128, 32), dtype=np.float16),
        })
    return in_maps


_NC_CACHE = {}


def _get_nc(S=S_FULL):
    if S not in _NC_CACHE:
        _NC_CACHE[S] = build_kernel(S)
    return _NC_CACHE[S]


def kernel(x, Wq, bq, Wk, bk, Wv, bv, Wo, bo, _trace=False, _trace_cores=None):
    from concourse import bass_utils

    nc = _get_nc(S_FULL)
    in_maps = shard_inputs(x, Wq, bq, Wk, bk, Wv, bv, Wo, bo)
    kwargs = {}
    if _trace:
        kwargs = dict(trace=True, trace_cores=_trace_cores or [0])
    res = bass_utils.run_bass_kernel_spmd(
        nc, in_maps, core_ids=list(range(NCORES)), **kwargs)
    out = np.zeros((B, S_FULL, D), dtype=np.float32)
    for c in range(NCORES):
        out += res.results[c]["part"]
    out += np.asarray(bo, dtype=np.float32)[None, None, :]
    if _trace:
        kernel._last_results = res
    return out


# revision 16
# speedup vs baseline: 1.4044x; 1.1505x over previous
"""Trainium2 Bass kernel for nn_MemoryEfficientAttention (full MHA).

Reference computation (fp32):
    q = split_heads(x @ Wq.T + bq); k, v likewise
    attn = softmax(q @ k.T / sqrt(64))
    out = merge_heads(attn @ v) @ Wo.T + bo

Shapes: B=2, S=4096, D=1024, H=16, head_dim=64.

Sharding across 8 NeuronCores (Megatron attention sharding):
  - 2 heads per core (= 128 of the 1024 projection dims, contiguous slice).
  - Q/K/V projections column-parallel, output projection row-parallel;
    the 8 per-core partial outputs are summed on the host (+ bo).
  - bv never enters the device: softmax rows sum to 1, so its entire effect
    on the output is the constant vector Wo @ bv, added on the host.

Per-core kernel (flash-attention style, nothing S^2-sized touches HBM):
  Phase 1: qT/kT = W_c @ x.T + b_c as fp16 matmuls ([128, S] transposed
           layouts); V projected directly in natural [S, 128] layout
           (x-tile stationary), stored with an interleaved ones-column per
           head (v_aug) that makes the PV matmul accumulate the softmax
           denominator in row 64 for free.
  Phase 2: per (batch, q-chunk of 512): loop over 32 key tiles:
           scoresT[kpos, q] for both heads (row-packed in the PE array,
           they run concurrently), exp on ScalarE (no max subtraction --
           scores are bounded by ~4, the softmax is mathematically
           identical), fp16 PV matmul with M=65. Raw output + denominator
           are copied out of PSUM immediately (fast accumulator release);
           normalization (reciprocal of a DMA-broadcast denominator)
           happens once per batch off the critical path.
  Phase 3: out[s, :] = attnT_c.T @ WoT_c in fp32r (natural layout, clean
           DMA out).
"""

import sys

if "/opt/trn_rl_repo" not in sys.path:
    sys.path.insert(0, "/opt/trn_rl_repo")

import numpy as np

B = 2
S_FULL = 4096
D = 1024
H = 16
HD = 64
NCORES = 8
DC = 128          # head dims per core (2 heads x 64)
SCALE = 1.0 / 8.0  # 1/sqrt(64)


def build_kernel(S=S_FULL):
    """Build the per-core Bass program. Returns the compiled Bacc object."""
    import concourse.bacc as bacc
    import concourse.tile as tile
    from concourse import mybir

    f32 = mybir.dt.float32
    f32r = mybir.dt.float32r
    f16 = mybir.dt.float16
    AF = mybir.ActivationFunctionType

    KT = D // 128       # k-tiles over the projection contraction dim
    SQ = 512            # q-chunk size
    NQC = S // SQ       # q chunks per batch
    NKT = S // 128      # key tiles per batch
    NM = S // 512       # x chunks for projections

    nc = bacc.Bacc("TRN2", target_bir_lowering=False, debug=False,
                   num_devices=NCORES)

    xT = nc.dram_tensor("xT", [B, D, S], f16, kind="ExternalInput").ap()
    wqT = nc.dram_tensor("wqT", [D, DC], f16, kind="ExternalInput").ap()
    wkT = nc.dram_tensor("wkT", [D, DC], f16, kind="ExternalInput").ap()
    wvT = nc.dram_tensor("wvT", [D, DC], f16, kind="ExternalInput").ap()
    woT = nc.dram_tensor("woT", [DC, D], f32r, kind="ExternalInput").ap()
    bq = nc.dram_tensor("bq", [DC], f32, kind="ExternalInput").ap()
    bk = nc.dram_tensor("bk", [DC], f32, kind="ExternalInput").ap()
    ones = nc.dram_tensor("ones", [128, 32], f16, kind="ExternalInput").ap()
    part = nc.dram_tensor("part", [B, S, D], f32, kind="ExternalOutput").ap()

    with tile.TileContext(nc) as tc:
        with (
            tc.tile_pool(name="consts", bufs=1) as consts,
            tc.tile_pool(name="xt", bufs=2) as xt_pool,
            tc.tile_pool(name="qkv", bufs=2) as qkv_pool,
            tc.tile_pool(name="exp", bufs=3) as exp_pool,
            tc.tile_pool(name="att", bufs=2) as att_pool,
            tc.tile_pool(name="small", bufs=2) as small_pool,
            tc.tile_pool(name="outs", bufs=3) as out_pool,
            tc.tile_pool(name="bounce", bufs=2, space="DRAM") as dram_pool,
            tc.tile_pool(name="ps_proj", bufs=2, space="PSUM") as ps_proj,
            tc.tile_pool(name="ps_scores", bufs=2, space="PSUM") as ps_scores,
            tc.tile_pool(name="ps_acc", bufs=2, space="PSUM") as ps_acc,
        ):
            # ---- constants ----
            wq_sb = consts.tile([128, KT, DC], f16)
            wk_sb = consts.tile([128, KT, DC], f16)
            wv_sb = consts.tile([128, KT, DC], f16)
            wo_sb = consts.tile([128, D], f32r)
            bq_sb = consts.tile([128, 1], f32)
            bk_sb = consts.tile([128, 1], f32)

            for w_sb, w_dram in ((wq_sb, wqT), (wk_sb, wkT), (wv_sb, wvT)):
                nc.sync.dma_start(
                    out=w_sb[:],
                    in_=w_dram.rearrange("(kt p) m -> p kt m", p=128),
                )
            nc.sync.dma_start(out=wo_sb[:], in_=woT)
            for b_sb, b_dram in ((bq_sb, bq), (bk_sb, bk)):
                nc.sync.dma_start(out=b_sb[:], in_=b_dram.rearrange("(p o) -> p o", o=1))

            for b in range(B):
                # ================= Phase 1: projections =================
                qT_sb = qkv_pool.tile([128, S], f16, tag="qT")
                kT_sb = qkv_pool.tile([128, S], f16, tag="kT")
                # v natural layout, per key-tile: [vA(64) | 1 | vB(64) | 1]
                v_sb = qkv_pool.tile([128, NKT, 130], f16, tag="v")
                ones_src = ones[:, 0:NKT].rearrange("p (t o) -> p t o", o=1)
                nc.sync.dma_start(out=v_sb[:, :, 64:65], in_=ones_src)
                nc.sync.dma_start(out=v_sb[:, :, 129:130], in_=ones_src)

                for m in range(NM):
                    xt = xt_pool.tile([128, KT, 512], f16)
                    nc.sync.dma_start(
                        out=xt[:],
                        in_=xT[b][:, m * 512:(m + 1) * 512].rearrange(
                            "(kt p) s -> p kt s", p=128),
                    )
                    for w_sb, b_sb, dst in (
                        (wq_sb, bq_sb, qT_sb),
                        (wk_sb, bk_sb, kT_sb),
                    ):
                        ps = ps_proj.tile([128, 512], f32, tag="proj")
                        for j in range(KT):
                            nc.tensor.matmul(
                                ps[:],
                                lhsT=w_sb[:, j, :],
                                rhs=xt[:, j, :],
                                start=(j == 0),
                                stop=(j == KT - 1),
                            )
                        nc.vector.tensor_scalar_add(
                            dst[:, m * 512:(m + 1) * 512], ps[:], b_sb[:],
                        )
                    # V in natural layout: x-tile stationary, Wv moving.
                    for t in range(4):
                        psv = ps_proj.tile([128, 512], f32, tag="proj")
                        for j in range(KT):
                            nc.tensor.matmul(
                                psv[:, 0:DC],
                                lhsT=xt[:, j, t * 128:(t + 1) * 128],
                                rhs=wv_sb[:, j, :],
                                start=(j == 0),
                                stop=(j == KT - 1),
                            )
                        kt_idx = m * 4 + t
                        nc.vector.tensor_copy(
                            v_sb[:, kt_idx, 0:64], psv[:, 0:64])
                        nc.vector.tensor_copy(
                            v_sb[:, kt_idx, 65:129], psv[:, 64:128])

                # ================= Phase 2: attention =================
                attT_sb = att_pool.tile([128, S], f32r, tag="attT")
                den_dram = dram_pool.tile([2, S], f32, tag="den")
                for qc in range(NQC):
                    q0, q1 = qc * SQ, (qc + 1) * SQ
                    acc_a = ps_acc.tile([128, SQ], f32, tag="acc")
                    acc_b = ps_acc.tile([128, SQ], f32, tag="acc")
                    accs = [acc_a, acc_b]
                    for j in range(NKT):
                        k0, k1 = j * 128, (j + 1) * 128
                        pss = ps_scores.tile([128, 2 * SQ], f32, tag="scores")
                        for hh in range(2):
                            nc.tensor.matmul(
                                pss[:, hh * SQ:(hh + 1) * SQ],
                                lhsT=kT_sb[hh * 64:(hh + 1) * 64, k0:k1],
                                rhs=qT_sb[hh * 64:(hh + 1) * 64, q0:q1],
                                start=True, stop=True,
                            )
                        ex = exp_pool.tile([128, 2 * SQ], f16, tag="exp")
                        nc.scalar.activation(ex[:], pss[:], AF.Exp, scale=SCALE)
                        for hh in range(2):
                            nc.tensor.matmul(
                                accs[hh][0:65, :],
                                lhsT=v_sb[:, j, hh * 65:(hh + 1) * 65],
                                rhs=ex[:, hh * SQ:(hh + 1) * SQ],
                                start=(j == 0), stop=(j == NKT - 1),
                            )
                    # fast PSUM release: copy raw output + denominator out
                    for hh in range(2):
                        nc.vector.tensor_copy(
                            attT_sb[hh * 64:(hh + 1) * 64, q0:q1],
                            accs[hh][0:64, :],
                        )
                        dn = small_pool.tile([1, SQ], f32, tag="dn")
                        nc.vector.tensor_copy(dn[:], accs[hh][64:65, :])
                        nc.sync.dma_start(out=den_dram[hh, q0:q1], in_=dn[:])

                # batch-level normalization, off the accumulation path
                NBC = 2048
                for hh in range(2):
                    for ch in range(S // NBC if S >= NBC else 1):
                        c0, c1 = ch * NBC, min((ch + 1) * NBC, S)
                        bc = small_pool.tile([128, min(NBC, S)], f32, tag="bcast")
                        bch = bc[hh * 64:(hh + 1) * 64, :]
                        rd = den_dram[hh, c0:c1]
                        bcast_src = rd.__class__(
                            tensor=rd.tensor, offset=rd.offset,
                            ap=[[0, 64]] + list(rd.ap),
                        )
                        nc.sync.dma_start(out=bch, in_=bcast_src)
                        nc.vector.reciprocal(bch, bch)
                        nc.vector.tensor_mul(
                            attT_sb[hh * 64:(hh + 1) * 64, c0:c1],
                            attT_sb[hh * 64:(hh + 1) * 64, c0:c1],
                            bch,
                        )

                # ================= Phase 3: output projection =================
                for st in range(S // 128):
                    s0, s1 = st * 128, (st + 1) * 128
                    for oc in range(D // 512):
                        pso = ps_acc.tile([128, 512], f32, tag="acc")
                        nc.tensor.matmul(
                            pso[:],
                            lhsT=attT_sb[:, s0:s1],
                            rhs=wo_sb[:, oc * 512:(oc + 1) * 512],
                            start=True, stop=True,
                        )
                        ob = out_pool.tile([128, 512], f32, tag="ob")
                        nc.vector.tensor_copy(ob[:], pso[:])
                        nc.sync.dma_start(
                            out=part[b, s0:s1, oc * 512:(oc + 1) * 512],
                            in_=ob[:],
                        )

    nc.compile()
    return nc


def shard_inputs(x, Wq, bq, Wk, bk, Wv, bv, Wo, bo, S=S_FULL):
    """Host-side sharding: returns list of 8 per-core input dicts."""
    x = np.asarray(x, dtype=np.float32)
    xT = np.ascontiguousarray(x.transpose(0, 2, 1)).astype(np.float16)  # [B, D, S]
    in_maps = []
    for c in range(NCORES):
        sl = slice(c * DC, (c + 1) * DC)
        in_maps.append({
            "xT": xT,
            "wqT": np.ascontiguousarray(np.asarray(Wq)[sl, :].T).astype(np.float16),
            "wkT": np.ascontiguousarray(np.asarray(Wk)[sl, :].T).astype(np.float16),
            "wvT": np.ascontiguousarray(np.asarray(Wv)[sl, :].T).astype(np.float16),
            "woT": np.ascontiguousarray(np.asarray(Wo)[:, sl].T, dtype=np.float32),
            "bq": np.ascontiguousarray(np.asarray(bq)[sl], dtype=np.float32),
            "bk": np.ascontiguousarray(np.asarray(bk)[sl], dtype=np.float32),
            "ones": np.ones((128, 32), dtype=np.float16),
        })
    return in_maps


_NC_CACHE = {}


def _get_nc(S=S_FULL):
    if S not in _NC_CACHE:
        _NC_CACHE[S] = build_kernel(S)
    return _NC_CACHE[S]


def kernel(x, Wq, bq, Wk, bk, Wv, bv, Wo, bo, _trace=False, _trace_cores=None):
    from concourse import bass_utils

    nc = _get_nc(S_FULL)
    in_maps = shard_inputs(x, Wq, bq, Wk, bk, Wv, bv, Wo, bo)
    kwargs = {}
    if _trace:
        kwargs = dict(trace=True, trace_cores=_trace_cores or [0])
    res = bass_utils.run_bass_kernel_spmd(
        nc, in_maps, core_ids=list(range(NCORES)), **kwargs)
    out = np.zeros((B, S_FULL, D), dtype=np.float32)
    for c in range(NCORES):
        out += res.results[c]["part"]
    # bv is folded out of the device kernel: softmax rows sum to one, so its
    # contribution to the output is the constant Wo @ bv. Add it with bo here.
    bias = (np.asarray(Wo, dtype=np.float64) @ np.asarray(bv, dtype=np.float64)
            + np.asarray(bo, dtype=np.float64))
    out += bias.astype(np.float32)[None, None, :]
    if _trace:
        kernel._last_results = res
    return out
